# revision 2
# baseline (speedup 1.0000x reference)
"""MoE-LoRA double GEMM on 8 Trainium2 NeuronCores.

Computes, for E=4 experts:  h_e = x @ A_e^T ; y_e = h_e @ B_e^T
with x:[4,2048,4096] f32, A:[4,64,4096], B:[4,4096,64] ->
y:[4,4,2048,4096] f32.

Strategy: data-parallel shard x over tokens (8192 tokens -> 1024/core),
replicate the small expert weights. The kernel is HBM-bandwidth bound
(~92 MB/core in fp32), so all device I/O runs in fp16: the host rounds
x/A/B to fp16 (matmul accumulation stays fp32 in PSUM; total rel err
~5e-4, far under the 2e-2 gate) and y comes back as fp16 which the
host upcasts. That halves HBM traffic to ~48 MB/core (~134 us at the
360 GB/s/core DMA roofline).

Device layouts are matmul-native, prepared on host:
  GEMM1: h^T[pair] = [A_2p^T | A_2p+1^T] (stationary, experts packed on
         the M axis) x x^T tile (moving, N=512) accumulated over D.
  GEMM2: y_e tile [128 tok, 512 out] = h_e^T chunk (stationary, K=64,
         the two experts of a pair on row strips 0/64) x B_e^T
         (moving), giving y in natural [token, out] layout for
         contiguous DMA stores.
GEMM2 results land in 2-bank PSUM tiles (both experts of a pair) so
one strided copy converts both to fp16; the copies rotate across
VectorE/ScalarE/PoolE so no single engine gates the store stream.
"""

import os
import sys

import numpy as np

for _p in ("/opt/trn_rl_repo", "/root/.axon_site/_ro/trn_rl_repo"):
    if os.path.isdir(_p) and _p not in sys.path:
        sys.path.append(_p)

from concourse import bacc, mybir, tile
from concourse.bass_utils import run_bass_kernel_spmd

E = 4
R_E = 64
D = 4096
O = 4096
B_DIM = 4
S = 2048
T = B_DIM * S          # 8192 tokens total
NCORES = 8
TL = T // NCORES       # 1024 tokens per core
TTS = [512, 512]       # pipeline tile sizes; sum must equal TL
NCD = D // 128         # 32 contraction chunks
OC_W = 512             # output columns per matmul (one PSUM bank, fp32)
NOC = O // OC_W        # 8

FP32 = mybir.dt.float32
FP16 = mybir.dt.float16

_CACHE = {}


def _build_nc():
    nc = bacc.Bacc(None, target_bir_lowering=False, debug=False)
    xt_d = nc.declare_dram_parameter("xT", [len(TTS), D, max(TTS)], FP16, isOutput=False)
    at_d = nc.declare_dram_parameter("AT", [2, 128, NCD, 128], FP16, isOutput=False)
    bt_d = nc.declare_dram_parameter("BT", [2, 128, O], FP16, isOutput=False)
    y_d = nc.declare_dram_parameter("y", [E, TL, O], FP16, isOutput=True)

    with tile.TileContext(nc) as tc:
        with (
            tc.tile_pool(name="atc", bufs=8) as atpool,
            tc.tile_pool(name="btc", bufs=2) as btpool,
            tc.tile_pool(name="xt", bufs=22) as xtpool,
            tc.tile_pool(name="ht", bufs=4) as htpool,
            tc.tile_pool(name="ys", bufs=4) as yspool,
            tc.tile_pool(name="ps_ht", bufs=2, space="PSUM") as ps_ht,
            tc.tile_pool(name="ps_y", bufs=3, space="PSUM") as ps_y,
        ):
            # PSUM->SBUF fp16 conversion copies rotate across three engines
            copy_engs = [
                nc.vector.tensor_copy,
                lambda dst, src: nc.scalar.copy(dst, src),
                nc.gpsimd.tensor_copy,
            ]

            # weight loads first (GEMM1 needs AT from chunk 0; BT before
            # the first GEMM2), then the x stream, all on the ScalarE ring
            # (SyncE carries the store stream)
            atq = [[None] * (NCD // 4) for _ in range(2)]
            btc = [None, None]
            for p in range(2):
                for k in range(NCD // 4):
                    ac = atpool.tile([128, 4, 128], FP16, name=f"at{p}_{k}", tag="atc")
                    nc.scalar.dma_start(out=ac[:], in_=at_d[p, :, 4 * k : 4 * k + 4, :])
                    atq[p][k] = ac
            for p in range(2):
                bc = btpool.tile([128, O], FP16, name=f"bt{p}", tag="btc")
                nc.scalar.dma_start(out=bc[:], in_=bt_d[p])
                btc[p] = bc
            xqs = []
            for tt, TTi in enumerate(TTS):
                xq = []
                for k in range(NCD // 2):
                    xc = xtpool.tile(
                        [128, 2, TTi], FP16, name=f"xc{tt}_{k}", tag="xtc"
                    )
                    nc.scalar.dma_start(
                        out=xc[:],
                        in_=xt_d[tt].rearrange("(c p) t -> p c t", p=128)[
                            :, 2 * k : 2 * k + 2, :TTi
                        ],
                    )
                    xq.append(xc)
                xqs.append(xq)

            t0 = 0
            ncopy = 0
            for tt, TTi in enumerate(TTS):
                TGi = TTi // 128
                xq = xqs[tt]
                phts = [
                    ps_ht.tile([128, TTi], FP32, name=f"pht{tt}_{_p}", tag="pht")
                    for _p in range(2)
                ]
                for c in range(NCD):
                    for p in range(2):
                        nc.tensor.matmul(
                            phts[p][:],
                            atq[p][c // 4][:, c % 4, :],
                            xq[c // 2][:, c % 2, :],
                            start=(c == 0),
                            stop=(c == NCD - 1),
                        )
                hts = []
                for p in range(2):
                    ht = htpool.tile([128, TTi], FP16, name=f"ht{tt}_{p}", tag="ht")
                    nc.vector.tensor_copy(ht[:], phts[p][:])
                    hts.append(ht)

                for p in range(2):
                    for g in range(TGi):
                        for qi in range(NOC // 4):
                            ysq = yspool.tile(
                                [128, 2, 4, OC_W],
                                FP16,
                                name=f"ys{tt}_{p}_{g}_{qi}",
                                tag="ys",
                            )
                            for j in range(4):
                                oc = 4 * qi + j
                                py = ps_y.tile([128, 2, OC_W], FP32)
                                for s_i in range(2):
                                    r0 = 64 * s_i
                                    nc.tensor.matmul(
                                        py[:, s_i, :],
                                        hts[p][
                                            r0 : r0 + 64, g * 128 : (g + 1) * 128
                                        ],
                                        btc[p][
                                            r0 : r0 + 64, oc * OC_W : (oc + 1) * OC_W
                                        ],
                                        start=True,
                                        stop=True,
                                    )
                                copy_engs[ncopy % 3](ysq[:, :, j, :], py[:])
                                ncopy += 1
                            for s_i in range(2):
                                e = 2 * p + s_i
                                nc.sync.dma_start(
                                    out=y_d[
                                        e,
                                        t0 + g * 128 : t0 + (g + 1) * 128,
                                        qi * 4 * OC_W : (qi + 1) * 4 * OC_W,
                                    ],
                                    in_=ysq[:, s_i],
                                )
                t0 += TTi
    nc.compile()
    return nc


def _get_nc():
    if "nc" not in _CACHE:
        _CACHE["nc"] = _build_nc()
    return _CACHE["nc"]


def _prep_weights(A, B):
    A = np.asarray(A, dtype=np.float32)
    B = np.asarray(B, dtype=np.float32)
    at = np.empty((2, 128, NCD, 128), dtype=np.float16)
    bt = np.empty((2, 128, O), dtype=np.float16)
    for p in range(2):
        # stationary for GEMM1: [D, 128] with expert 2p in cols 0-63, 2p+1 in 64-127
        atp = np.concatenate([A[2 * p].T, A[2 * p + 1].T], axis=1)  # [4096, 128]
        at[p] = atp.reshape(NCD, 128, 128).transpose(1, 0, 2)
        # moving for GEMM2: [128, O] with expert 2p rows 0-63, 2p+1 rows 64-127
        bt[p] = np.concatenate([B[2 * p].T, B[2 * p + 1].T], axis=0)
    return at, bt


def kernel(x, A, B, _trace=False):
    x = np.asarray(x, dtype=np.float32)
    xt_full = x.reshape(T, D).T  # [D, T] view
    at, bt = _prep_weights(A, B)

    nc = _get_nc()
    in_maps = []
    for k in range(NCORES):
        xh = np.empty((len(TTS), D, max(TTS)), dtype=np.float16)
        t0 = 0
        for tt, TTi in enumerate(TTS):
            xh[tt, :, :TTi] = xt_full[:, k * TL + t0 : k * TL + t0 + TTi]
            t0 += TTi
        in_maps.append({"xT": xh, "AT": at, "BT": bt})
    res = run_bass_kernel_spmd(nc, in_maps, list(range(NCORES)), trace=_trace)
    if _trace:
        _CACHE["last_result"] = res

    y = np.empty((E, T, O), dtype=np.float32)
    for k in range(NCORES):
        y[:, k * TL : (k + 1) * TL, :] = res.results[k]["y"]
    return y.reshape(E, B_DIM, S, O)
